# revision 38
# baseline (speedup 1.0000x reference)
"""Multi-head attention + output projection on 8 Trainium2 NeuronCores.

Problem (hardcoded): B=2, N=S=2048, DIM=1024, 8 heads, head_dim=128, fp32.
  out = softmax(Q K^T / sqrt(128)) V  -> reshape -> @ proj_w.T + proj_b

Sharding: data parallel on batch (2) x tensor parallel on heads (4 groups of
2 heads).  Each core computes attention for its 2 heads plus the partial
output projection restricted to its heads' columns; the host sums the 4
partial projections per batch (fp16 partials upcast on host) and adds bias.

Per-core kernel (matmul operands fp16, accumulation fp32 PSUM):
  S^T = K @ Q^T per 128-row s-chunk with s on partitions; exp on ScalarE
  (PSUM->SBUF, scale pre-applied to Q on host); out^T = V^T @ expS^T
  accumulated in PSUM.  Row sums: fp16 tree adds on DVE + a ones-matmul
  colsum; reciprocal on DVE is hoisted off the o_ps critical path so only
  the final normalize multiply gates the projection.

  Engine split: TensorE matmuls, ScalarE exp, DVE rowsum tree + normalize +
  reciprocal, GpSimd (Pool) evacuates projection PSUM -> fp16 SBUF, DMA
  writes fp16 partials per 128-row chunk as soon as they are ready.

  Emission is software-pipelined for the in-order queues: PV lags 4 groups
  behind QK/exp (2 for the very first head, which has no deferred work to
  hide behind), and tail work is deferred into the next head's stream via a
  work queue drained 2-3 closures per group.
"""

import sys
from collections import deque

sys.path.insert(0, "/opt/trn_rl_repo")

import numpy as np

import concourse.bass as bass  # noqa: F401  (engine namespaces live on nc)
import concourse.mybir as mybir
import concourse.tile as tile
from concourse import bacc
from concourse.bass_utils import run_bass_kernel_spmd

B = 2
N = 2048
S = 2048
DIM = 1024
NUM_HEADS = 8
HD = 128
N_CORES = 8
HEADS_PER_CORE = 2  # 4-way head parallel x 2-way batch parallel
HG = DIM // (NUM_HEADS // HEADS_PER_CORE)  # 256 dims per core
P = 128
SC = S // P  # 16 s-chunks
NB = 512  # query-column block
NQ = N // NB
GC = 2  # s-chunks per exp group
NG = SC // GC  # 8 groups per (head, block)
F32 = mybir.dt.float32
F16 = mybir.dt.float16

_nc_cache = {}


def _build():
    nc = bacc.Bacc(None, target_bir_lowering=False, debug=False, num_devices=1)

    qt = nc.dram_tensor("qt", [HG, N], F16, kind="ExternalInput").ap()
    kt = nc.dram_tensor("kt", [HG, S], F16, kind="ExternalInput").ap()
    v = nc.dram_tensor("v", [S, HG], F16, kind="ExternalInput").ap()
    wt = nc.dram_tensor("wt", [HG, DIM], F16, kind="ExternalInput").ap()
    out = nc.dram_tensor("out", [N, DIM], F16, kind="ExternalOutput").ap()

    EXPF = mybir.ActivationFunctionType.Exp

    with tile.TileContext(nc) as tc:
        with (
            tc.tile_pool(name="persist", bufs=1) as persist,
            tc.tile_pool(name="e_pool", bufs=10) as e_pool,
            tc.tile_pool(name="a_pool", bufs=3) as a_pool,
            tc.tile_pool(name="small", bufs=3) as small,
            tc.tile_pool(name="y_pool", bufs=3) as y_pool,
            tc.tile_pool(name="s_ps_pool", bufs=2, space="PSUM") as s_ps_pool,
            tc.tile_pool(name="acc_ps_pool", bufs=4, space="PSUM") as acc_ps_pool,
        ):
            # Resident inputs, sliced so the first QK group starts early.
            qt_sb = persist.tile([P, HEADS_PER_CORE, N], F16)
            kt_sb = persist.tile([P, HEADS_PER_CORE, S], F16)
            v_sb = persist.tile([P, HEADS_PER_CORE, SC, HD], F16)
            wt_sb = persist.tile([P, HEADS_PER_CORE, DIM], F16)
            qt_r = qt.rearrange("(h p) n -> p h n", p=P)
            kt_r = kt.rearrange("(h p) s -> p h s", p=P)
            v_r = v.rearrange("(c p) (h d) -> p h c d", p=P, h=HEADS_PER_CORE)
            wt_r = wt.rearrange("(h p) o -> p h o", p=P)
            ones_dram = nc.inline_tensor(np.ones((P, P), np.float16), name="ones_const")
            ones_mat = persist.tile([P, P], F16)
            # Critical path: first QK needs kt chunks 0-1 + q block 0, and the
            # first head runs PV at lag 2 so v chunks 0-3 come right after.
            # Bulk h1 loads issue in parallel from the GpSimd queue.
            nc.sync.dma_start(out=kt_sb[:, 0, 0 : 4 * P], in_=kt_r[:, 0, 0 : 4 * P])
            # qt issues from the Scalar queue in parallel with kt on Sync;
            # Scalar has nothing queued before its ACT_TABLE_LOAD/first exp
            nc.scalar.dma_start(out=qt_sb[:, 0, 0:NB], in_=qt_r[:, 0, 0:NB])
            nc.sync.dma_start(
                out=kt_sb[:, 0, 4 * P : S // 2], in_=kt_r[:, 0, 4 * P : S // 2]
            )
            nc.sync.dma_start(out=v_sb[:, 0, 0:4], in_=v_r[:, 0, 0:4])
            nc.sync.dma_start(out=kt_sb[:, 0, S // 2 :], in_=kt_r[:, 0, S // 2 :])
            nc.sync.dma_start(out=v_sb[:, 0, 4:], in_=v_r[:, 0, 4:])
            nc.sync.dma_start(out=ones_mat, in_=ones_dram.ap())
            nc.sync.dma_start(out=qt_sb[:, 0, NB:], in_=qt_r[:, 0, NB:])
            nc.sync.dma_start(out=kt_sb[:, 1], in_=kt_r[:, 1])
            nc.sync.dma_start(out=qt_sb[:, 1], in_=qt_r[:, 1])
            nc.sync.dma_start(out=v_sb[:, 1], in_=v_r[:, 1])
            nc.sync.dma_start(out=wt_sb, in_=wt_r)

            # X^T: normalized attention outputs, head-dim on partitions.
            xt_sb = persist.tile([P, HEADS_PER_CORE, N], F16)

            # PE clock warm-up: the PE starts at ~0.65-1.2GHz and reaches
            # 2.4GHz only after ~3us of busy time.  Burn the ramp on dummy
            # matmuls during the initial DMA wait.
            scratch = persist.tile([P, NB], F16)
            nc.vector.memset(scratch, 0)
            warm_ps = acc_ps_pool.tile([P, NB], F32, tag="acc")
            for _ in range(3):
                nc.tensor.matmul(
                    warm_ps, scratch[:, 0:P], scratch, start=True, stop=True
                )

            def pv_pair(o_ps, h, e_t, g):
                def fn():
                    for j in range(GC):
                        si = GC * g + j
                        nc.tensor.matmul(
                            o_ps, v_sb[:, h, si, :], e_t[:, j, :],
                            start=(si == 0), stop=(si == SC - 1),
                        )
                return fn

            def add_one(acc, src):
                def fn():
                    with nc.allow_low_precision(reason="fp16 rowsum partials"):
                        nc.vector.tensor_add(acc, acc, src)
                return fn

            def rowsum_vh(rb_ps, a2, a2g, recip):
                # Rowsum + reciprocal; depends only on e-tiles, not o_ps.
                def fn():
                    with nc.allow_low_precision(reason="fp16 rowsum partials"):
                        nc.vector.tensor_add(a2, a2, a2g)
                        nc.vector.tensor_add(a2[:, 0, :], a2[:, 0, :], a2[:, 1, :])
                    nc.tensor.matmul(rb_ps, ones_mat, a2[:, 0, :], start=True, stop=True)
                    nc.vector.reciprocal_approx_fast(out=recip, in_=rb_ps)
                return fn

            def norm_vh(o_ps, recip, h, nsl):
                def fn():
                    with nc.allow_low_precision(reason="fp16 attention output grid"):
                        nc.vector.tensor_mul(xt_sb[:, h, nsl], o_ps, recip)
                return fn

            def proj_half(nq, t, ot, y_sb):
                def fn():
                    nt = nq * (NB // P) + t
                    y_ps = acc_ps_pool.tile([P, NB], F32, tag="acc")
                    for hh in range(HEADS_PER_CORE):
                        nc.tensor.matmul(
                            y_ps,
                            xt_sb[:, hh, nt * P : (nt + 1) * P],
                            wt_sb[:, hh, ot * NB : (ot + 1) * NB],
                            start=(hh == 0),
                            stop=(hh == HEADS_PER_CORE - 1),
                        )
                    if nq == NQ - 1 and ot == 0:
                        # final block: ScalarE is idle after its last exp --
                        # split the evacuation across engines to shorten the
                        # end-of-kernel CAST chain
                        nc.scalar.activation(
                            out=y_sb[:, t, ot * NB : (ot + 1) * NB],
                            in_=y_ps,
                            func=mybir.ActivationFunctionType.Copy,
                        )
                    else:
                        with nc.allow_low_precision(
                            reason="fp16 partial projection"
                        ):
                            nc.vector.tensor_copy(
                                y_sb[:, t, ot * NB : (ot + 1) * NB], y_ps
                            )
                    if ot == 1:
                        r0 = nq * NB + t * P
                        nc.sync.dma_start(
                            out=out[r0 : r0 + P, :], in_=y_sb[:, t, :]
                        )
                return fn

            work_q = deque()
            for nq in range(NQ):
                nsl = slice(nq * NB, (nq + 1) * NB)
                for h in range(HEADS_PER_CORE):
                    first = nq == 0 and h == 0
                    LAG = 2 if first else 4
                    q_blk = qt_sb[:, h, nsl]
                    o_ps = acc_ps_pool.tile([P, NB], F32, tag="acc")
                    a2 = a_pool.tile([P, GC, NB], F16, tag="a2")
                    a2g = a_pool.tile([P, GC, NB], F16, tag="a2g")
                    rb_ps = None
                    es = []  # exp tiles in flight
                    for g in range(NG):
                        s_ps = s_ps_pool.tile([P, GC, NB], F32, tag="s")
                        for j in range(GC):
                            si = GC * g + j
                            nc.tensor.matmul(
                                s_ps[:, j, :],
                                kt_sb[:, h, si * P : (si + 1) * P],
                                q_blk,
                                start=True,
                                stop=True,
                            )
                        e_t = e_pool.tile([P, GC, NB], F16, tag="e")
                        nc.scalar.activation(out=e_t, in_=s_ps, func=EXPF)
                        es.append(e_t)

                        # drain deferred work: 2 closures per group
                        for _ in range(2):
                            if work_q:
                                work_q.popleft()()

                        # PV + rowsum accumulation lag LAG groups behind exp
                        if g >= LAG:
                            pg = g - LAG
                            pe = es[pg]
                            for j in range(GC):
                                si = GC * pg + j
                                nc.tensor.matmul(
                                    o_ps, v_sb[:, h, si, :], pe[:, j, :],
                                    start=(si == 0), stop=False,
                                )
                            if pg == 2:
                                with nc.allow_low_precision(
                                    reason="fp16 rowsum partials; r ~2e3, ~3e-4 rel"
                                ):
                                    nc.vector.tensor_add(a2, es[0], es[2])
                            elif pg == 3:
                                with nc.allow_low_precision(
                                    reason="fp16 rowsum partials"
                                ):
                                    nc.vector.tensor_add(a2g, es[1], es[3])
                            elif pg >= 4:
                                acc = a2 if pg % 2 == 0 else a2g
                                with nc.allow_low_precision(
                                    reason="fp16 rowsum partials"
                                ):
                                    nc.vector.tensor_add(acc, acc, es[pg])
                        if g == NG - 1:
                            rb_ps = acc_ps_pool.tile([P, NB], F32, tag="acc")
                    # defer PV of the last LAG groups, remaining folds, rowsum,
                    # and normalize into the next head's stream
                    recip = small.tile([P, NB], F32, tag="recip")
                    tail_pgs = list(range(NG - LAG, NG))
                    for pg in tail_pgs[:-1]:
                        work_q.append(pv_pair(o_ps, h, es[pg], pg))
                        if pg >= 4:
                            work_q.append(add_one(a2 if pg % 2 == 0 else a2g, es[pg]))
                    last = tail_pgs[-1]
                    work_q.append(add_one(a2 if last % 2 == 0 else a2g, es[last]))
                    # rowsum+recip hoisted before the last PV pair (they only
                    # need the folded e-tiles) so just the normalize multiply
                    # gates on o_ps completion
                    work_q.append(rowsum_vh(rb_ps, a2, a2g, recip))
                    work_q.append(pv_pair(o_ps, h, es[last], last))
                    work_q.append(norm_vh(o_ps, recip, h, nsl))
                y_sb = y_pool.tile([P, NB // P, DIM], F16, tag="y")
                for t in range(NB // P):
                    for ot in range(2):
                        work_q.append(proj_half(nq, t, ot, y_sb))

            while work_q:
                work_q.popleft()()

    nc.compile()
    return nc


def kernel(query, key, value, proj_w, proj_b):
    query = np.asarray(query)
    key = np.asarray(key)
    value = np.asarray(value)
    proj_w = np.asarray(proj_w)
    proj_b = np.asarray(proj_b)
    if "nc" not in _nc_cache:
        _nc_cache["nc"] = _build()
    nc = _nc_cache["nc"]

    scale = float(HD) ** -0.5
    wt_full = np.ascontiguousarray(proj_w.T.astype(np.float32))  # [in, out]
    in_maps = []
    for core in range(N_CORES):
        b, hg = divmod(core, N_CORES // B)
        sl = slice(hg * HG, (hg + 1) * HG)
        in_maps.append(
            {
                "qt": np.ascontiguousarray((query[b].T[sl] * scale), dtype=np.float16),
                "kt": np.ascontiguousarray(key[b].T[sl], dtype=np.float16),
                "v": np.ascontiguousarray(value[b][:, sl], dtype=np.float16),
                "wt": np.ascontiguousarray(wt_full[sl], dtype=np.float16),
            }
        )

    res = run_bass_kernel_spmd(nc, in_maps, list(range(N_CORES)))

    out = np.zeros((B, N, DIM), dtype=np.float32)
    for core in range(N_CORES):
        b = core // (N_CORES // B)
        out[b] += res.results[core]["out"].astype(np.float32)
    out += proj_b.astype(np.float32)
    return out
